# revision 9
# baseline (speedup 1.0000x reference)
"""Llama GQA causal attention layer (the "topk" in the module name is a
mathematical identity) on 8 Trainium2 NeuronCores.

Sharding: tensor-parallel over heads. Each core owns 2 of the 16 q-heads and
the single kv-head they share, computes its slice of Q/K/V projections, RoPE,
causal flash attention (scores kept on-chip in [k, q] orientation), and a
row-slice o_proj producing a full-shape [S, HID] partial; the host sums the 8
partials.

v2 design (vs the 404us v1 baseline):
  - Feature-major QKV projection: lhsT = weight tiles, rhs = hidden^T chunks,
    so q^T/k^T come out of PSUM directly in [hd, token] orientation -- no PE
    transposes or evac copies for q/k.
  - RoPE applied in [hd, token] layout with partition-offset DVE ops against
    full-width CC/SS tables (4 ops per 512-token tile).
  - v transposed to token-major via DMA-xbar transpose (off-engine).
  - exp batched across the 2 heads: scores land in a [128, 2, 512] 2-bank
    PSUM pair, one ACTIVATE covers both heads (144 ops instead of 288).
  - Softmax denominator accumulated as an fp16 [128, 2, 512] running sum on
    DVE (tensor_add per k-tile), reduced by one ones-column matmul per head.
  - Separate PSUM pools (scores-pairs 4 banks / att 2 / proj 2) so the
    readiness-based Tile scheduler can always slot projection matmuls into
    PE gaps left by the QK->exp->AV dependency chain.
  - o_proj emitted at the chunk boundary (PSUM pair pool is idle there).

Shapes hardcoded per problem spec:
  hidden_states [1, 4096, 2048] f32, position_ids [1, 4096] i32,
  Wq [2048, 2048], Wk/Wv [2048, 512], Wo [2048, 2048] f32.
"""

import math
import sys

import numpy as np

if "/opt/trn_rl_repo" not in sys.path:
    sys.path.insert(0, "/opt/trn_rl_repo")

import concourse.bass as bass
import concourse.mybir as mybir
import concourse.tile as tile
from concourse import bacc, bass_utils

B, S, HID = 1, 4096, 2048
NH, KVH, HD = 16, 4, 128
GROUPS = NH // KVH
NCORES = 8
HPC = NH // NCORES          # q heads per core = 2
ST = S // 128               # 32 s-tiles
KT = HID // 128             # 16 hid-tiles (contraction)
QCH = 512                   # q chunk width for attention
NQC = S // QCH
ROPE_THETA = 10000.0
ISQ = 1.0 / math.sqrt(HD)

F32 = mybir.dt.float32
F16 = mybir.dt.float16
EXP = mybir.ActivationFunctionType.Exp


def build_body(tc, out, ht, wqkv, wo, ccd, ssd, trimask2d, onesd):
    """DRAM layouts (host pre-arranged, partition dim first, fp16):
      ht    [128, KT, S]    ht[p, t, s] = hidden[s, 128 t + p]
      wqkv  [128, KT, 512]  wqkv[p,t,j] = [Wq_c | Wk_c | Wv_c][128 t + p, j]
      wo    [128, HPC, HID] wo[p, j, n] = Wo[256 c + 128 j + p, n]
      ccd   [128, S]        ccd[p, s] = cos(s * invfreq_{p%64})
      ssd   [128, S]        ssd[p, s] = +sin(..) for p<64, -sin for p>=64
                            (row p multiplies raw[(p+64)%128]; the t1 output
                            lands on the opposite half, so signs are swapped)
      trimask2d [128, 2, 128]  k-major causal mask, duplicated along j
      onesd [128, 1] fp16 ones column
      out   [S, HID] fp16 partial output (sum over cores on host)
    """
    nc = tc.nc

    with (
        tc.tile_pool(name="const", bufs=1) as constp,
        tc.tile_pool(name="slabs", bufs=1) as slabs,
        tc.tile_pool(name="hsp", bufs=3) as hsp,
        tc.tile_pool(name="qtp", bufs=2) as qtp,
        tc.tile_pool(name="ropep", bufs=2) as rp,
        tc.tile_pool(name="ptb", bufs=4) as ptb,
        tc.tile_pool(name="accp", bufs=2) as accp,
        tc.tile_pool(name="atp", bufs=2) as atp,
        tc.tile_pool(name="denb", bufs=2) as denb,
        tc.tile_pool(name="csb", bufs=2) as cp,
        tc.tile_pool(name="sp", bufs=2, space="PSUM") as spp,
        tc.tile_pool(name="attps", bufs=2, space="PSUM") as attp,
        tc.tile_pool(name="pqps", bufs=2, space="PSUM") as pqp,
    ):
        # ---- constants ----
        # wqkv rides the sync queue interleaved with the first hs chunk so
        # chunk 0's projection can start ASAP; everything else goes on the
        # scalar engine's DMA queue.
        hs_tiles = {}

        def prefetch_hs(qc, queue=None):
            q = queue or nc.sync
            hs = hsp.tile([128, KT, QCH], F16, tag="hs", name=f"hs{qc}")
            for sl in range(0, QCH, 128):
                q.dma_start(
                    out=hs[:, :, sl : sl + 128],
                    in_=ht[:, :, qc * QCH + sl : qc * QCH + sl + 128],
                )
            hs_tiles[qc] = hs

        wqkv_sb = constp.tile([128, KT, 512], F16)
        hs0 = hsp.tile([128, KT, QCH], F16, tag="hs")
        for t in range(KT):
            nc.sync.dma_start(out=wqkv_sb[:, t, :], in_=wqkv[:, t, :])
            nc.sync.dma_start(out=hs0[:, t, :], in_=ht[:, t, 0:QCH])
        hs_tiles[0] = hs0

        cc_sb = constp.tile([128, S], F16)
        nc.scalar.dma_start(out=cc_sb[:, 0:QCH], in_=ccd[:, 0:QCH])
        ss_sb = constp.tile([128, S], F16)
        nc.scalar.dma_start(out=ss_sb[:, 0:QCH], in_=ssd[:, 0:QCH])
        trimask2 = constp.tile([128, 2, 128], F16)
        nc.scalar.dma_start(out=trimask2, in_=trimask2d)
        ones_col = constp.tile([128, 1], F16)
        nc.scalar.dma_start(out=ones_col, in_=onesd)
        nc.scalar.dma_start(out=cc_sb[:, QCH:], in_=ccd[:, QCH:])
        nc.scalar.dma_start(out=ss_sb[:, QCH:], in_=ssd[:, QCH:])
        wo_sb = constp.tile([128, HPC, HID], F16)
        nc.scalar.dma_start(out=wo_sb, in_=wo)

        kTc, vc = {}, {}
        qts = {}
        state = {}

        def stage_A(qc):
            """Feature-major projection + rope for chunk qc."""
            hs = hs_tiles.pop(qc)
            q0 = qc * QCH
            qt = qtp.tile([128, HPC, QCH], F16, tag="qt", name=f"qt{qc}")
            kt_c = slabs.tile([128, QCH], F16, tag=f"kT{qc}", name=f"kT{qc}")
            v_c = slabs.tile([128, QCH], F16, tag=f"v{qc}", name=f"v{qc}")
            kTc[qc], vc[qc] = kt_c, v_c
            qts[qc] = qt
            for j in range(4):
                pq = pqp.tile([128, QCH], F32, tag="pq", name=f"pq{qc}_{j}")
                for t in range(KT):
                    nc.tensor.matmul(
                        pq,
                        lhsT=wqkv_sb[:, t, 128 * j : 128 * j + 128],
                        rhs=hs[:, t, :],
                        start=(t == 0),
                        stop=(t == KT - 1),
                    )
                if j < 3:
                    # rope: dst = raw*CC + swaphalf(raw)*SS
                    raw = rp.tile([128, QCH], F16, tag="raw", name=f"raw{qc}_{j}")
                    nc.scalar.copy(raw, pq)
                    dst = qt[:, j, :] if j < 2 else kt_c
                    t1 = rp.tile([128, QCH], F16, tag="t1", name=f"t1_{qc}_{j}")
                    t2 = rp.tile([128, QCH], F16, tag="t2", name=f"t2_{qc}_{j}")
                    nc.vector.tensor_mul(
                        t1[0:64, :], raw[64:128, :], ss_sb[64:128, q0 : q0 + QCH]
                    )
                    nc.vector.tensor_mul(
                        t1[64:128, :], raw[0:64, :], ss_sb[0:64, q0 : q0 + QCH]
                    )
                    nc.vector.tensor_mul(t2, raw, cc_sb[:, q0 : q0 + QCH])
                    nc.vector.tensor_add(dst, t1, t2)
                else:
                    # v: copy out then DMA-xbar transpose to token-major
                    rawv = rp.tile([128, QCH], F16, tag="rawv", name=f"rawv{qc}")
                    nc.scalar.copy(rawv, pq)
                    for i in range(4):
                        nc.sync.dma_start(
                            out=v_c[:, 128 * i : 128 * i + 128],
                            in_=rawv[:, 128 * i : 128 * i + 128],
                            transpose=True,
                        )

        def stage_B(qc):
            """Causal attention for chunk qc: scores in [k, q], both heads
            share each 2-bank score pair; fp16 running den accumulator."""
            qt = qts.pop(qc)
            q0 = qc * QCH
            nkt = 4 * qc + 4
            att = [
                attp.tile([128, QCH], F32, tag="att", name=f"att{qc}_{h}")
                for h in range(HPC)
            ]
            acc = accp.tile([128, HPC, QCH], F16, tag="acc", name=f"acc{qc}")
            prev = None
            for kt in range(nkt):
                k0 = kt * 128
                off = max(0, k0 - q0)
                sp = spp.tile([128, HPC, QCH], F32, tag="sp", name=f"sp{qc}_{kt}")
                for h in range(HPC):
                    nc.tensor.matmul(
                        sp[:, h, off:],
                        lhsT=kTc[kt // 4][:, 128 * (kt % 4) : 128 * (kt % 4) + 128],
                        rhs=qt[:, h, off:],
                        start=True,
                        stop=True,
                    )
                pt = ptb.tile([128, HPC, QCH], F16, tag="pt", name=f"pt{qc}_{kt}")
                nc.scalar.activation(pt[:, :, off:], sp[:, :, off:], EXP, scale=ISQ)
                if kt >= 4 * qc:
                    nc.vector.tensor_mul(
                        pt[:, :, off : off + 128], pt[:, :, off : off + 128], trimask2
                    )
                if kt == 0:
                    nc.vector.tensor_copy(acc, pt)
                else:
                    nc.vector.tensor_add(acc[:, :, off:], acc[:, :, off:], pt[:, :, off:])
                if prev is not None:
                    emit_av(att, prev, False)
                prev = (kt, off, pt)
            emit_av(att, prev, True)
            state[qc] = (att, acc)

        def emit_av(att, item, is_last):
            kt, off, pt = item
            for h in range(HPC):
                nc.tensor.matmul(
                    att[h][:, off:],
                    lhsT=vc[kt // 4][:, 128 * (kt % 4) : 128 * (kt % 4) + 128],
                    rhs=pt[:, h, off:],
                    start=(kt == 0),
                    stop=is_last,
                )

        def stage_den(qc):
            att, acc = state.pop(qc)
            at_c = atp.tile([128, HPC, QCH], F16, tag="at", name=f"at{qc}")
            dd = spp.tile([128, HPC, QCH], F32, tag="sp", name=f"dd{qc}")
            rden = denb.tile([1, HPC, QCH], F32, tag="rden", name=f"rden{qc}")
            rdb = denb.tile([128, HPC, QCH], F32, tag="rdb", name=f"rdb{qc}")
            for h in range(HPC):
                nc.tensor.matmul(
                    dd[0:1, h, :], lhsT=ones_col, rhs=acc[:, h, :], start=True, stop=True
                )
                nc.vector.reciprocal_approx_fast(rden[:, h, :], dd[0:1, h, :])
                nc.gpsimd.partition_broadcast(rdb[:, h, :], rden[:, h, :])
                nc.vector.tensor_mul(at_c[:, h, :], att[h], rdb[:, h, :])
            return at_c

        def stage_C(qc, at_c):
            for si in range(4):
                s0 = (4 * qc + si) * 128
                sl = si * 128
                osb = cp.tile([128, HID], F16, tag="osb", name=f"osb{qc}_{si}")
                osbv = osb.rearrange("p (c n) -> p c n", n=512)
                for nchp in range(2):
                    po = spp.tile(
                        [128, 2, 512], F32, tag="sp", name=f"po{qc}_{si}_{nchp}"
                    )
                    for j in range(HPC):
                        for b in range(2):
                            n0 = (2 * nchp + b) * 512
                            nc.tensor.matmul(
                                po[:, b, :],
                                lhsT=at_c[:, j, sl : sl + 128],
                                rhs=wo_sb[:, j, n0 : n0 + 512],
                                start=(j == 0),
                                stop=(j == HPC - 1),
                            )
                    # alternate the PSUM->SBUF evac between ACT and DVE to
                    # balance the two engines
                    if nchp == 0:
                        nc.scalar.copy(osbv[:, 0:2, :], po)
                    else:
                        nc.vector.tensor_copy(osbv[:, 2:4, :], po)
                nc.sync.dma_start(out=out[s0 : s0 + 128, :], in_=osb)

        # ---- pipeline ----
        # Emission order = scheduler priority. The readiness-based scheduler
        # slots stage_A(qc+1) projection matmuls into PE gaps left by
        # B(qc)'s QK->exp->AV chain (separate PSUM pools make them ready).
        # C(qc-1) is emitted after den(qc)/A(qc+1): its PSUM-pair pool slots
        # come up right when B(qc)'s scores drain, so its o_proj matmuls fill
        # the chunk boundary while den(qc)'s recip/broadcast chain resolves.
        stage_A(0)
        prefetch_hs(1)
        at_cs = {}
        for qc in range(NQC):
            if qc + 2 < NQC:
                prefetch_hs(qc + 2)
            stage_B(qc)
            at_cs[qc] = stage_den(qc)
            if qc + 1 < NQC:
                stage_A(qc + 1)
            if qc > 0:
                stage_C(qc - 1, at_cs.pop(qc - 1))
        stage_C(NQC - 1, at_cs.pop(NQC - 1))


_NC_CACHE = {}


def get_nc():
    key = "nc"
    if key not in _NC_CACHE:
        nc = bacc.Bacc(
            "TRN2",
            debug=False,
            enable_asserts=False,
            target_bir_lowering=False,
        )
        ht = nc.dram_tensor("ht", [128, KT, S], F16, kind="ExternalInput").ap()
        wqkv = nc.dram_tensor("wqkv", [128, KT, 512], F16, kind="ExternalInput").ap()
        wo = nc.dram_tensor("wo", [128, HPC, HID], F16, kind="ExternalInput").ap()
        ccd = nc.dram_tensor("ccd", [128, S], F16, kind="ExternalInput").ap()
        ssd = nc.dram_tensor("ssd", [128, S], F16, kind="ExternalInput").ap()
        trimask2d = nc.dram_tensor(
            "trimask2d", [128, 2, 128], F16, kind="ExternalInput"
        ).ap()
        onesd = nc.dram_tensor("onesd", [128, 1], F16, kind="ExternalInput").ap()
        out = nc.dram_tensor("out", [S, HID], F16, kind="ExternalOutput").ap()
        with tile.TileContext(nc) as tc:
            build_body(tc, out, ht, wqkv, wo, ccd, ssd, trimask2d, onesd)
        nc.compile()
        _NC_CACHE[key] = nc
    return _NC_CACHE[key]


def prep_in_maps(hidden_states, position_ids, Wq, Wk, Wv, Wo):
    hid = np.asarray(hidden_states, dtype=np.float32)[0]          # [S, HID]
    pos = np.asarray(position_ids)[0].astype(np.float32)          # [S]
    Wq = np.asarray(Wq, dtype=np.float32)
    Wk = np.asarray(Wk, dtype=np.float32)
    Wv = np.asarray(Wv, dtype=np.float32)
    Wo = np.asarray(Wo, dtype=np.float32)

    inv = 1.0 / (ROPE_THETA ** (np.arange(0, HD, 2, dtype=np.float32) / HD))
    freqs = pos[:, None] * inv[None, :]                           # [S, 64]
    cos_r = np.cos(freqs).T                                        # [64, S]
    sin_r = np.sin(freqs).T
    cc = np.concatenate([cos_r, cos_r], axis=0).astype(np.float16)  # [128, S]
    ss = np.concatenate([sin_r, -sin_r], axis=0).astype(np.float16)
    ht_r = np.ascontiguousarray(
        hid.T.reshape(KT, 128, S).transpose(1, 0, 2)
    ).astype(np.float16)
    tri = np.triu(np.ones((128, 128), np.float16))
    trimask2 = np.ascontiguousarray(np.repeat(tri[:, None, :], 2, axis=1))

    in_maps = []
    for c in range(NCORES):
        kv = c // 2
        wqkv_c = np.concatenate(
            [
                Wq[:, 256 * c : 256 * (c + 1)],
                Wk[:, 128 * kv : 128 * (kv + 1)],
                Wv[:, 128 * kv : 128 * (kv + 1)],
            ],
            axis=1,
        )                                                          # [2048, 512]
        wqkv_r = np.ascontiguousarray(
            wqkv_c.reshape(KT, 128, 512).transpose(1, 0, 2)
        ).astype(np.float16)
        wo_r = np.ascontiguousarray(
            Wo[256 * c : 256 * (c + 1), :].reshape(HPC, 128, HID).transpose(1, 0, 2)
        ).astype(np.float16)
        in_maps.append(
            {
                "ht": ht_r,
                "wqkv": wqkv_r,
                "wo": wo_r,
                "ccd": cc,
                "ssd": ss,
                "trimask2d": trimask2,
                "onesd": np.ones((128, 1), np.float16),
            }
        )
    return in_maps


def run_spmd(in_maps, **kw):
    nc = get_nc()
    return bass_utils.run_bass_kernel_spmd(
        nc, in_maps, core_ids=list(range(NCORES)), **kw
    )


def kernel(hidden_states, position_ids, Wq, Wk, Wv, Wo):
    in_maps = prep_in_maps(hidden_states, position_ids, Wq, Wk, Wv, Wo)
    res = run_spmd(in_maps)
    total = res.results[0]["out"].astype(np.float32)
    for c in range(1, NCORES):
        total = total + res.results[c]["out"].astype(np.float32)
    return total[None]


# revision 10
# speedup vs baseline: 1.1069x; 1.1069x over previous
"""Llama GQA causal attention layer (the "topk" in the module name is a
mathematical identity) on 8 Trainium2 NeuronCores.

Sharding: tensor-parallel over heads. Each core owns 2 of the 16 q-heads and
the single kv-head they share, computes its slice of Q/K/V projections, RoPE,
causal flash attention (scores kept on-chip in [k, q] orientation), and a
row-slice o_proj producing a full-shape [S, HID] partial; the host sums the 8
partials.

v2 design (vs the 404us v1 baseline):
  - Feature-major QKV projection: lhsT = weight tiles, rhs = hidden^T chunks,
    so q^T/k^T come out of PSUM directly in [hd, token] orientation -- no PE
    transposes or evac copies for q/k.
  - RoPE applied in [hd, token] layout with partition-offset DVE ops against
    full-width CC/SS tables (4 ops per 512-token tile).
  - v transposed to token-major via DMA-xbar transpose (off-engine).
  - exp batched across the 2 heads: scores land in a [128, 2, 512] 2-bank
    PSUM pair, one ACTIVATE covers both heads (144 ops instead of 288).
  - Softmax denominator accumulated as an fp16 [128, 2, 512] running sum on
    DVE (tensor_add per k-tile), reduced by one ones-column matmul per head.
  - Separate PSUM pools (scores-pairs 4 banks / att 2 / proj 2) so the
    readiness-based Tile scheduler can always slot projection matmuls into
    PE gaps left by the QK->exp->AV dependency chain.
  - o_proj emitted at the chunk boundary (PSUM pair pool is idle there).

Shapes hardcoded per problem spec:
  hidden_states [1, 4096, 2048] f32, position_ids [1, 4096] i32,
  Wq [2048, 2048], Wk/Wv [2048, 512], Wo [2048, 2048] f32.
"""

import math
import sys

import numpy as np

if "/opt/trn_rl_repo" not in sys.path:
    sys.path.insert(0, "/opt/trn_rl_repo")

import concourse.bass as bass
import concourse.mybir as mybir
import concourse.tile as tile
from concourse import bacc, bass_utils

B, S, HID = 1, 4096, 2048
NH, KVH, HD = 16, 4, 128
GROUPS = NH // KVH
NCORES = 8
HPC = NH // NCORES          # q heads per core = 2
ST = S // 128               # 32 s-tiles
KT = HID // 128             # 16 hid-tiles (contraction)
QCH = 512                   # q chunk width for attention
NQC = S // QCH
ROPE_THETA = 10000.0
ISQ = 1.0 / math.sqrt(HD)

F32 = mybir.dt.float32
F16 = mybir.dt.float16
EXP = mybir.ActivationFunctionType.Exp


def build_body(tc, out, ht, wqkv, wo, ccd, ssd, trimask2d, onesd):
    """DRAM layouts (host pre-arranged, partition dim first, fp16):
      ht    [128, KT, S]    ht[p, t, s] = hidden[s, 128 t + p]
      wqkv  [128, KT, 512]  wqkv[p,t,j] = [Wq_c | Wk_c | Wv_c][128 t + p, j]
      wo    [128, HPC, HID] wo[p, j, n] = Wo[256 c + 128 j + p, n]
      ccd   [128, S]        ccd[p, s] = cos(s * invfreq_{p%64})
      ssd   [128, S]        ssd[p, s] = +sin(..) for p<64, -sin for p>=64
                            (row p multiplies raw[(p+64)%128]; the t1 output
                            lands on the opposite half, so signs are swapped)
      trimask2d [128, 2, 128]  k-major causal mask, duplicated along j
      onesd [128, 1] fp16 ones column
      out   [S, HID] fp16 partial output (sum over cores on host)
    """
    nc = tc.nc

    with (
        tc.tile_pool(name="const", bufs=1) as constp,
        tc.tile_pool(name="slabs", bufs=1) as slabs,
        tc.tile_pool(name="hsp", bufs=3) as hsp,
        tc.tile_pool(name="qtp", bufs=2) as qtp,
        tc.tile_pool(name="ropep", bufs=2) as rp,
        tc.tile_pool(name="ptb", bufs=4) as ptb,
        tc.tile_pool(name="accp", bufs=2) as accp,
        tc.tile_pool(name="atp", bufs=2) as atp,
        tc.tile_pool(name="denb", bufs=2) as denb,
        tc.tile_pool(name="csb", bufs=2) as cp,
        tc.tile_pool(name="sp", bufs=2, space="PSUM") as spp,
        tc.tile_pool(name="attps", bufs=2, space="PSUM") as attp,
        tc.tile_pool(name="pqps", bufs=2, space="PSUM") as pqp,
    ):
        # ---- constants ----
        # wqkv rides the sync queue interleaved with the first hs chunk so
        # chunk 0's projection can start ASAP; everything else goes on the
        # scalar engine's DMA queue.
        hs_tiles = {}

        def prefetch_hs(qc, queue=None):
            q = queue or nc.sync
            hs = hsp.tile([128, KT, QCH], F16, tag="hs", name=f"hs{qc}")
            for sl in range(0, QCH, 128):
                q.dma_start(
                    out=hs[:, :, sl : sl + 128],
                    in_=ht[:, :, qc * QCH + sl : qc * QCH + sl + 128],
                )
            hs_tiles[qc] = hs

        wqkv_sb = constp.tile([128, KT, 512], F16)
        hs0 = hsp.tile([128, KT, QCH], F16, tag="hs")
        for t in range(KT):
            nc.sync.dma_start(out=wqkv_sb[:, t, :], in_=wqkv[:, t, :])
            nc.sync.dma_start(out=hs0[:, t, :], in_=ht[:, t, 0:QCH])
        hs_tiles[0] = hs0

        cc_sb = constp.tile([128, S], F16)
        nc.scalar.dma_start(out=cc_sb[:, 0:QCH], in_=ccd[:, 0:QCH])
        ss_sb = constp.tile([128, S], F16)
        nc.scalar.dma_start(out=ss_sb[:, 0:QCH], in_=ssd[:, 0:QCH])
        trimask2 = constp.tile([128, 2, 128], F16)
        nc.scalar.dma_start(out=trimask2, in_=trimask2d)
        ones_col = constp.tile([128, 1], F16)
        nc.scalar.dma_start(out=ones_col, in_=onesd)
        nc.scalar.dma_start(out=cc_sb[:, QCH:], in_=ccd[:, QCH:])
        nc.scalar.dma_start(out=ss_sb[:, QCH:], in_=ssd[:, QCH:])
        wo_sb = constp.tile([128, HPC, HID], F16)
        nc.scalar.dma_start(out=wo_sb, in_=wo)

        kTc, vc = {}, {}
        qts = {}
        state = {}

        def stage_A(qc):
            """Feature-major projection + rope for chunk qc."""
            hs = hs_tiles.pop(qc)
            q0 = qc * QCH
            qt = qtp.tile([128, HPC, QCH], F16, tag="qt", name=f"qt{qc}")
            kt_c = slabs.tile([128, QCH], F16, tag=f"kT{qc}", name=f"kT{qc}")
            v_c = slabs.tile([128, QCH], F16, tag=f"v{qc}", name=f"v{qc}")
            kTc[qc], vc[qc] = kt_c, v_c
            qts[qc] = qt
            for j in range(4):
                pq = pqp.tile([128, QCH], F32, tag="pq", name=f"pq{qc}_{j}")
                for t in range(KT):
                    nc.tensor.matmul(
                        pq,
                        lhsT=wqkv_sb[:, t, 128 * j : 128 * j + 128],
                        rhs=hs[:, t, :],
                        start=(t == 0),
                        stop=(t == KT - 1),
                    )
                if j < 3:
                    # rope: dst = raw*CC + swaphalf(raw)*SS
                    raw = rp.tile([128, QCH], F16, tag="raw", name=f"raw{qc}_{j}")
                    nc.scalar.copy(raw, pq)
                    dst = qt[:, j, :] if j < 2 else kt_c
                    t1 = rp.tile([128, QCH], F16, tag="t1", name=f"t1_{qc}_{j}")
                    t2 = rp.tile([128, QCH], F16, tag="t2", name=f"t2_{qc}_{j}")
                    nc.vector.tensor_mul(
                        t1[0:64, :], raw[64:128, :], ss_sb[64:128, q0 : q0 + QCH]
                    )
                    nc.vector.tensor_mul(
                        t1[64:128, :], raw[0:64, :], ss_sb[0:64, q0 : q0 + QCH]
                    )
                    nc.vector.tensor_mul(t2, raw, cc_sb[:, q0 : q0 + QCH])
                    nc.vector.tensor_add(dst, t1, t2)
                else:
                    # v: copy out then DMA-xbar transpose to token-major
                    rawv = rp.tile([128, QCH], F16, tag="rawv", name=f"rawv{qc}")
                    nc.scalar.copy(rawv, pq)
                    for i in range(4):
                        nc.sync.dma_start(
                            out=v_c[:, 128 * i : 128 * i + 128],
                            in_=rawv[:, 128 * i : 128 * i + 128],
                            transpose=True,
                        )

        def stage_B(qc):
            """Causal attention for chunk qc: scores in [k, q], both heads
            share each 2-bank score pair; fp16 running den accumulator."""
            qt = qts.pop(qc)
            q0 = qc * QCH
            nkt = 4 * qc + 4
            att = [
                attp.tile([128, QCH], F32, tag="att", name=f"att{qc}_{h}")
                for h in range(HPC)
            ]
            acc = accp.tile([128, HPC, QCH], F16, tag="acc", name=f"acc{qc}")
            prev = None
            for kt in range(nkt):
                k0 = kt * 128
                off = max(0, k0 - q0)
                sp = spp.tile([128, HPC, QCH], F32, tag="sp", name=f"sp{qc}_{kt}")
                for h in range(HPC):
                    nc.tensor.matmul(
                        sp[:, h, off:],
                        lhsT=kTc[kt // 4][:, 128 * (kt % 4) : 128 * (kt % 4) + 128],
                        rhs=qt[:, h, off:],
                        start=True,
                        stop=True,
                    )
                pt = ptb.tile([128, HPC, QCH], F16, tag="pt", name=f"pt{qc}_{kt}")
                nc.scalar.activation(pt[:, :, off:], sp[:, :, off:], EXP, scale=ISQ)
                if kt >= 4 * qc:
                    nc.vector.tensor_mul(
                        pt[:, :, off : off + 128], pt[:, :, off : off + 128], trimask2
                    )
                if kt == 0:
                    nc.vector.tensor_copy(acc, pt)
                else:
                    nc.vector.tensor_add(acc[:, :, off:], acc[:, :, off:], pt[:, :, off:])
                if prev is not None:
                    emit_av(att, prev, False)
                prev = (kt, off, pt)
            emit_av(att, prev, True)
            state[qc] = (att, acc)

        def emit_av(att, item, is_last):
            kt, off, pt = item
            for h in range(HPC):
                nc.tensor.matmul(
                    att[h][:, off:],
                    lhsT=vc[kt // 4][:, 128 * (kt % 4) : 128 * (kt % 4) + 128],
                    rhs=pt[:, h, off:],
                    start=(kt == 0),
                    stop=is_last,
                )

        def stage_den(qc):
            att, acc = state.pop(qc)
            at_c = atp.tile([128, HPC, QCH], F16, tag="at", name=f"at{qc}")
            dd = spp.tile([128, HPC, QCH], F32, tag="sp", name=f"dd{qc}")
            rden = denb.tile([1, HPC, QCH], F32, tag="rden", name=f"rden{qc}")
            rdb = denb.tile([128, HPC, QCH], F32, tag="rdb", name=f"rdb{qc}")
            for h in range(HPC):
                nc.tensor.matmul(
                    dd[0:1, h, :], lhsT=ones_col, rhs=acc[:, h, :], start=True, stop=True
                )
                nc.vector.reciprocal_approx_fast(rden[:, h, :], dd[0:1, h, :])
                nc.gpsimd.partition_broadcast(rdb[:, h, :], rden[:, h, :])
                nc.vector.tensor_mul(at_c[:, h, :], att[h], rdb[:, h, :])
            return at_c

        def stage_C(qc, at_c):
            for si in range(4):
                s0 = (4 * qc + si) * 128
                sl = si * 128
                osb = cp.tile([128, HID], F16, tag="osb", name=f"osb{qc}_{si}")
                osbv = osb.rearrange("p (c n) -> p c n", n=512)
                for nchp in range(2):
                    po = spp.tile(
                        [128, 2, 512], F32, tag="sp", name=f"po{qc}_{si}_{nchp}"
                    )
                    for j in range(HPC):
                        for b in range(2):
                            n0 = (2 * nchp + b) * 512
                            nc.tensor.matmul(
                                po[:, b, :],
                                lhsT=at_c[:, j, sl : sl + 128],
                                rhs=wo_sb[:, j, n0 : n0 + 512],
                                start=(j == 0),
                                stop=(j == HPC - 1),
                            )
                    # alternate the PSUM->SBUF evac between ACT and DVE to
                    # balance the two engines
                    if nchp == 0:
                        nc.scalar.copy(osbv[:, 0:2, :], po)
                    else:
                        nc.vector.tensor_copy(osbv[:, 2:4, :], po)
                nc.sync.dma_start(out=out[s0 : s0 + 128, :], in_=osb)

        # ---- pipeline ----
        # Emission order = scheduler priority. The readiness-based scheduler
        # slots stage_A(qc+1) projection matmuls into PE gaps left by
        # B(qc)'s QK->exp->AV chain (separate PSUM pools make them ready).
        # C(qc-1) is emitted after den(qc)/A(qc+1): its PSUM-pair pool slots
        # come up right when B(qc)'s scores drain, so its o_proj matmuls fill
        # the chunk boundary while den(qc)'s recip/broadcast chain resolves.
        stage_A(0)
        prefetch_hs(1)
        at_cs = {}
        for qc in range(NQC):
            if qc + 2 < NQC:
                prefetch_hs(qc + 2)
            stage_B(qc)
            at_cs[qc] = stage_den(qc)
            if qc > 0:
                stage_C(qc - 1, at_cs.pop(qc - 1))
            if qc + 1 < NQC:
                stage_A(qc + 1)
        stage_C(NQC - 1, at_cs.pop(NQC - 1))


_NC_CACHE = {}


def get_nc():
    key = "nc"
    if key not in _NC_CACHE:
        nc = bacc.Bacc(
            "TRN2",
            debug=False,
            enable_asserts=False,
            target_bir_lowering=False,
        )
        ht = nc.dram_tensor("ht", [128, KT, S], F16, kind="ExternalInput").ap()
        wqkv = nc.dram_tensor("wqkv", [128, KT, 512], F16, kind="ExternalInput").ap()
        wo = nc.dram_tensor("wo", [128, HPC, HID], F16, kind="ExternalInput").ap()
        ccd = nc.dram_tensor("ccd", [128, S], F16, kind="ExternalInput").ap()
        ssd = nc.dram_tensor("ssd", [128, S], F16, kind="ExternalInput").ap()
        trimask2d = nc.dram_tensor(
            "trimask2d", [128, 2, 128], F16, kind="ExternalInput"
        ).ap()
        onesd = nc.dram_tensor("onesd", [128, 1], F16, kind="ExternalInput").ap()
        out = nc.dram_tensor("out", [S, HID], F16, kind="ExternalOutput").ap()
        with tile.TileContext(nc) as tc:
            build_body(tc, out, ht, wqkv, wo, ccd, ssd, trimask2d, onesd)
        nc.compile()
        _NC_CACHE[key] = nc
    return _NC_CACHE[key]


def prep_in_maps(hidden_states, position_ids, Wq, Wk, Wv, Wo):
    hid = np.asarray(hidden_states, dtype=np.float32)[0]          # [S, HID]
    pos = np.asarray(position_ids)[0].astype(np.float32)          # [S]
    Wq = np.asarray(Wq, dtype=np.float32)
    Wk = np.asarray(Wk, dtype=np.float32)
    Wv = np.asarray(Wv, dtype=np.float32)
    Wo = np.asarray(Wo, dtype=np.float32)

    inv = 1.0 / (ROPE_THETA ** (np.arange(0, HD, 2, dtype=np.float32) / HD))
    freqs = pos[:, None] * inv[None, :]                           # [S, 64]
    cos_r = np.cos(freqs).T                                        # [64, S]
    sin_r = np.sin(freqs).T
    cc = np.concatenate([cos_r, cos_r], axis=0).astype(np.float16)  # [128, S]
    ss = np.concatenate([sin_r, -sin_r], axis=0).astype(np.float16)
    ht_r = np.ascontiguousarray(
        hid.T.reshape(KT, 128, S).transpose(1, 0, 2)
    ).astype(np.float16)
    tri = np.triu(np.ones((128, 128), np.float16))
    trimask2 = np.ascontiguousarray(np.repeat(tri[:, None, :], 2, axis=1))

    in_maps = []
    for c in range(NCORES):
        kv = c // 2
        wqkv_c = np.concatenate(
            [
                Wq[:, 256 * c : 256 * (c + 1)],
                Wk[:, 128 * kv : 128 * (kv + 1)],
                Wv[:, 128 * kv : 128 * (kv + 1)],
            ],
            axis=1,
        )                                                          # [2048, 512]
        wqkv_r = np.ascontiguousarray(
            wqkv_c.reshape(KT, 128, 512).transpose(1, 0, 2)
        ).astype(np.float16)
        wo_r = np.ascontiguousarray(
            Wo[256 * c : 256 * (c + 1), :].reshape(HPC, 128, HID).transpose(1, 0, 2)
        ).astype(np.float16)
        in_maps.append(
            {
                "ht": ht_r,
                "wqkv": wqkv_r,
                "wo": wo_r,
                "ccd": cc,
                "ssd": ss,
                "trimask2d": trimask2,
                "onesd": np.ones((128, 1), np.float16),
            }
        )
    return in_maps


def run_spmd(in_maps, **kw):
    nc = get_nc()
    return bass_utils.run_bass_kernel_spmd(
        nc, in_maps, core_ids=list(range(NCORES)), **kw
    )


def kernel(hidden_states, position_ids, Wq, Wk, Wv, Wo):
    in_maps = prep_in_maps(hidden_states, position_ids, Wq, Wk, Wv, Wo)
    res = run_spmd(in_maps)
    total = res.results[0]["out"].astype(np.float32)
    for c in range(1, NCORES):
        total = total + res.results[c]["out"].astype(np.float32)
    return total[None]
